# revision 1
# baseline (speedup 1.0000x reference)
"""AutoCorrelation Trainium2 kernel (Bass/Tile, 8 NeuronCores).

Math (per row r of [B*L, 512] with D=512):
  corr_r = irfft(rfft(q_r) * conj(rfft(k_r)))            (circular cross-correlation)
  mean_r = mean(top7(corr_r))
  w0 = sigmoid(corr - mean); out = v*w0 + roll(v,-1,L)*(1-w0)
     = v + sigmoid(mean - corr) * (roll(v) - v)

Implementation:
  - DFT/iDFT as fp16 matmuls on the PE with a packed-real 512-point basis:
    packed[f] layout: A-block f=0..255 = Re[f] (A[0]=Re0), B-block = Im[f]
    (B[0]=Re256).  Forward: QF^T[fpacked, row] = W^T q^T via
    lhsT=W-block, rhs=qT (DMA-xbar-transposed q16).  Product spectrum
    P = QF o conj(KF) elementwise on DVE (block formulas + 2 f=0 fixups).
    Inverse: corr[row, t] via lhsT=P-chunk, rhs=C-block -> PSUM fp32,
    already in row-major layout.
  - top-7 mean via the DVE max8 instruction reading corr in PSUM.
  - sigmoid on ACT directly off PSUM with per-partition bias = +mean/scale=-1.
  - Row interleave: partition p = row//64, subblock s = row%64 makes
    roll(v,-1) = "read subblock s+1" (same partitions); batch wraps and the
    s=63 edge are handled by one small shifted DMA load (vsh).
  - Sharding: batch-parallel, 4 batches per core, no communication.
"""
import numpy as np

B, L, D = 32, 2048, 512
N_CORES = 8
BPC = B // N_CORES            # batches per core
ROWS = BPC * L                # 8192 rows per core
NSUB = 64                     # subblocks (s = row % 64)
P = 128                       # partitions (p = row // 64)
SB_GROUP = 8                  # subblocks per DMA superblock
NSUPER = NSUB // SB_GROUP     # 8 superblocks
TOPK = 7

_CACHE = {}


def _dft_consts():
    """Packed-real DFT matrices W [512 feat, 512 packed] and C [512 packed, 512 t]."""
    j = np.arange(D)[:, None].astype(np.float64)
    f = np.arange(256)[None, :].astype(np.float64)
    Wc = np.cos(-2 * np.pi * j * f / D)
    Ws = np.sin(-2 * np.pi * j * f / D)
    WB = Ws.copy()
    WB[:, 0] = np.cos(np.pi * j[:, 0])          # B0 row: Re256
    W = np.concatenate([Wc, WB], axis=1)        # [512, 512]
    t = np.arange(D)[None, :].astype(np.float64)
    fc = np.arange(256)[:, None].astype(np.float64)
    Ca = np.cos(2 * np.pi * fc * t / D) * 2 / D
    Ca[0] = 1.0 / D
    Cb = -np.sin(2 * np.pi * fc * t / D) * 2 / D
    Cb[0] = np.cos(np.pi * t[0]) / D
    C = np.concatenate([Ca, Cb], axis=0)        # [512, 512]
    return W.astype(np.float32), C.astype(np.float32)


def _build_nc(n_iter=1):
    import os
    import concourse.bacc as bacc
    import concourse.mybir as mybir
    from concourse.tile import TileContext

    ABL = set(os.environ.get("AUTOCORR_ABL", "").split(","))

    f16 = mybir.dt.float16
    f32 = mybir.dt.float32

    W, C = _dft_consts()
    # W16[p, jj, fp]  = W[jj*128+p, fp]   (lhsT blocks for GEMM-1)
    W16 = W.reshape(4, P, D).transpose(1, 0, 2).astype(np.float16).copy()
    # C16[p, ff, t]   = C[ff*128+p, t]    (rhs blocks for GEMM-2)
    C16 = C.reshape(4, P, D).transpose(1, 0, 2).astype(np.float16).copy()

    nc = bacc.Bacc()
    q_d = nc.dram_tensor("query", [ROWS, D], f32, kind="ExternalInput")
    k_d = nc.dram_tensor("key", [ROWS, D], f32, kind="ExternalInput")
    v_d = nc.dram_tensor("value", [ROWS, D], f32, kind="ExternalInput")
    o_d = nc.dram_tensor("out", [ROWS, D], f32, kind="ExternalOutput")
    w_t = nc.inline_tensor(W16, name="Wdft")
    c_t = nc.inline_tensor(C16, name="Cdft")

    # interleaved views: [p, s, c] with row = 64*p + s
    qv = q_d.rearrange("(p s) c -> p s c", s=NSUB)
    kv = k_d.rearrange("(p s) c -> p s c", s=NSUB)
    vv = v_d.rearrange("(p s) c -> p s c", s=NSUB)
    ov = o_d.rearrange("(p s) c -> p s c", s=NSUB)

    with TileContext(nc) as tc:
        with (
            tc.tile_pool(name="consts", bufs=1) as consts,
            tc.tile_pool(name="io", bufs=2) as io,
            tc.tile_pool(name="work", bufs=3) as work,
            tc.tile_pool(name="small", bufs=8) as small,
            tc.tile_pool(name="ps", bufs=3, space="PSUM") as psp,
            tc.tile_pool(name="pscb", bufs=2, space="PSUM") as pscp,
        ):
            wt = consts.tile([P, 4, D], f16)      # W16
            ct = consts.tile([P, 4, D], f16)      # C16
            nc.sync.dma_start(out=wt, in_=w_t[:, :, :])
            nc.sync.dma_start(out=ct, in_=c_t[:, :, :])

            # vsh[p] = v[row 64p+64] ; fix wraps at p in {31,63,95,127} <- batch starts
            vsh = consts.tile([P, D], f16)
            vflat = v_d  # [ROWS, D]
            nc.gpsimd.dma_start(
                out=vsh[0:127], in_=vflat.rearrange("(a b) c -> a b c", b=NSUB)[1:128, 0]
            )  # rows 64,128,...,8128
            nc.gpsimd.dma_start(
                out=vsh.rearrange("(w u) c -> w u c", u=32)[:, 31:32, :].rearrange("w u c -> (w u) c"),
                in_=vflat.rearrange("(b t) c -> b t c", t=L)[:, 0:1, :].rearrange("b t c -> (b t) c"),
            )  # vsh[31,63,95,127] <- v rows {0, 2048, 4096, 6144}

            def load_super(sbi):
                sl = slice(sbi * SB_GROUP, (sbi + 1) * SB_GROUP)
                q16 = io.tile([P, SB_GROUP, D], f16, tag="q16")
                k16 = io.tile([P, SB_GROUP, D], f16, tag="k16")
                v16 = io.tile([P, SB_GROUP, D], f16, tag="v16")
                if "loadhalf" in ABL:
                    nc.gpsimd.dma_start(out=q16, in_=qv[:, sl, :])
                    return q16, q16, q16
                nc.gpsimd.dma_start(out=q16, in_=qv[:, sl, :])
                nc.gpsimd.dma_start(out=k16, in_=kv[:, sl, :])
                nc.gpsimd.dma_start(out=v16, in_=vv[:, sl, :])
                return q16, k16, v16

            def compute_group(qT8, kT8, gl, w1sb):
                """gl: local group index (0..3) inside superblock; reads subblocks
                2gl, 2gl+1 from the whole-superblock transpose tiles qT8/kT8
                (mid index u*4+jj, u = local subblock); writes w1 into w1sb."""
                psq = psp.tile([P, 4, 256], f32, tag="ps2bank")
                psk = psp.tile([P, 4, 256], f32, tag="ps2bank")
                x0 = 8 * gl
                for mm in range(4):
                    for jj in range(4):
                        nc.tensor.matmul(psq[:, mm, :], wt[:, jj, mm * P:(mm + 1) * P],
                                         qT8[:, x0 + jj:x0 + jj + 5:4, :],
                                         start=(jj == 0), stop=(jj == 3))
                for mm in range(4):
                    for jj in range(4):
                        nc.tensor.matmul(psk[:, mm, :], wt[:, jj, mm * P:(mm + 1) * P],
                                         kT8[:, x0 + jj:x0 + jj + 5:4, :],
                                         start=(jj == 0), stop=(jj == 3))

                qf = work.tile([P, 4, 256], f16, tag="qf")
                kf = work.tile([P, 4, 256], f16, tag="kf")
                nc.scalar.copy(qf, psq)
                nc.scalar.copy(kf, psk)

                # products: Pa = QA.KA + QB.KB ; Pb = QB.KA - QA.KB
                pt = work.tile([P, 4, 256], f16, tag="pt")
                t1 = work.tile([P, 2, 256], f16, tag="t1")
                t2 = work.tile([P, 2, 256], f16, tag="t2")
                QA, QB = qf[:, 0:2, :], qf[:, 2:4, :]
                KA, KB = kf[:, 0:2, :], kf[:, 2:4, :]
                nc.vector.tensor_mul(t1, QA, KA)
                nc.vector.tensor_mul(t2, QB, KB)
                nc.vector.tensor_add(pt[:, 0:2, :], t1, t2)
                nc.vector.tensor_mul(t1, QB, KA)
                nc.vector.tensor_mul(t2, QA, KB)
                nc.vector.tensor_sub(pt[:, 2:4, :], t1, t2)
                # f=0 fixups (partition 0 of slices 0 and 2), one strided op
                nc.vector.tensor_mul(
                    pt[0:1, 0:4:2, :], qf[0:1, 0:4:2, :], kf[0:1, 0:4:2, :])

                for sp in range(2):
                    cps = pscp.tile([P, D], f32, tag="psc1bank")
                    for ff in range(4):
                        nc.tensor.matmul(cps, pt[:, ff, sp * P:(sp + 1) * P],
                                         ct[:, ff, :], start=(ff == 0), stop=(ff == 3))
                    mx = small.tile([P, 8], f32, tag="mx")
                    nc.vector.max(out=mx, in_=cps)
                    sm = small.tile([P, 1], f32, tag="sm")
                    nc.vector.reduce_sum(sm, mx[:, 0:TOPK], axis=mybir.AxisListType.X)
                    pm = small.tile([P, 1], f32, tag="pm")
                    nc.vector.tensor_scalar_mul(pm, sm, 1.0 / TOPK)
                    nc.scalar.activation(w1sb[:, 2 * gl + sp, :], cps,
                                         mybir.ActivationFunctionType.Sigmoid,
                                         bias=pm, scale=-1.0)

            def combine_super(v16, w1sb, vnext0, o16):
                """o16[:, s] = v16[:, s] + w1sb[:, s]*(v16[:, s+1] - v16[:, s]);
                s=7 uses vnext0."""
                for sl in range(SB_GROUP):
                    vnext = v16[:, sl + 1, :] if sl < SB_GROUP - 1 else vnext0
                    dt_ = work.tile([P, D], f16, tag="dt")
                    zt = work.tile([P, D], f16, tag="zt")
                    nc.vector.tensor_sub(dt_, vnext, v16[:, sl, :])
                    nc.vector.tensor_mul(zt, w1sb[:, sl, :], dt_)
                    nc.gpsimd.tensor_add(o16[:, sl, :], v16[:, sl, :], zt)

            def pipeline():
                prev = None  # (v16, o16, w1sb, sbi)
                for sbi in range(NSUPER):
                    q16, k16, v16 = load_super(sbi)
                    o16 = io.tile([P, SB_GROUP, D], f16, tag="o16")
                    w1sb = work.tile([P, SB_GROUP, D], f16, tag="w1sb", bufs=2)
                    qT8 = work.tile([P, 32, P], f16, tag="qT8", bufs=2)
                    kT8 = work.tile([P, 32, P], f16, tag="kT8", bufs=2)
                    nc.sync.dma_start_transpose(
                        qT8, q16.rearrange("p s c -> p (s c)"))
                    nc.sync.dma_start_transpose(
                        kT8, k16.rearrange("p s c -> p (s c)"))
                    for gl in range(4):
                        compute_group(qT8, kT8, gl, w1sb)
                    if prev is not None:
                        pv, po, pw, psbi = prev
                        combine_super(pv, pw, v16[:, 0, :], po)
                        nc.gpsimd.dma_start(
                            out=ov[:, psbi * SB_GROUP:(psbi + 1) * SB_GROUP, :], in_=po)
                    prev = (v16, o16, w1sb, sbi)

                pv, po, pw, psbi = prev
                combine_super(pv, pw, vsh, po)
                nc.gpsimd.dma_start(
                    out=ov[:, psbi * SB_GROUP:(psbi + 1) * SB_GROUP, :], in_=po)

            if n_iter == 1:
                pipeline()
            else:
                with tc.For_i(0, n_iter, 1):
                    pipeline()

    nc.finalize()
    return nc


def kernel(query, key, value):
    import sys
    if "/opt/trn_rl_repo" not in sys.path:
        sys.path.insert(0, "/opt/trn_rl_repo")
    from concourse.bass_utils import run_bass_kernel_spmd

    if "nc" not in _CACHE:
        _CACHE["nc"] = _build_nc()
    nc = _CACHE["nc"]

    q = np.ascontiguousarray(np.asarray(query, dtype=np.float32).reshape(B, L, D))
    k = np.ascontiguousarray(np.asarray(key, dtype=np.float32).reshape(B, L, D))
    v = np.ascontiguousarray(np.asarray(value, dtype=np.float32).reshape(B, L, D))

    in_maps = []
    for c in range(N_CORES):
        sl = slice(c * BPC, (c + 1) * BPC)
        in_maps.append({
            "query": q[sl].reshape(ROWS, D),
            "key": k[sl].reshape(ROWS, D),
            "value": v[sl].reshape(ROWS, D),
        })
    res = run_bass_kernel_spmd(nc, in_maps, core_ids=list(range(N_CORES)),
                               trace=bool(_CACHE.get("trace")))
    _CACHE["last_result"] = res
    out = np.empty((B, L, D), dtype=np.float32)
    for c in range(N_CORES):
        out[c * BPC:(c + 1) * BPC] = res.results[c]["out"].reshape(BPC, L, D)
    return out



# revision 2
# speedup vs baseline: 1.3713x; 1.3713x over previous
"""AutoCorrelation Trainium2 kernel (Bass/Tile, 8 NeuronCores) — v2.

Math (per row r of [B*L, 512] with D=512):
  corr_r = irfft(rfft(q_r) * conj(rfft(k_r)))            (circular cross-correlation)
  mean_r = mean(top7(corr_r))
  w0 = sigmoid(corr - mean); out = v*w0 + roll(v,-1,L)*(1-w0)
     = v + sigmoid(mean - corr) * (roll(v) - v)

v2 changes vs v1 (which used DMA-xbar transposes, all DMA on one ring, fp32 out):
  - q/k transposed on the PE (is_transpose matmuls, f16 chunks into PSUM) and
    copied PSUM->SBUF on ACT/Pool.  This removes ~33 MB/iter of SBUF<->SBUF
    xbar traffic from the DMA fabric (measured cap ~320 GB/s/core shared).
  - Output DRAM tensor is f16 (host converts to fp32): halves store traffic.
  - Input loads spread across the three DMA queues (SP/ACT/Pool).
"""
import numpy as np

B, L, D = 32, 2048, 512
N_CORES = 8
BPC = B // N_CORES            # batches per core
ROWS = BPC * L                # 8192 rows per core
NSUB = 64                     # subblocks (s = row % 64)
P = 128                       # partitions (p = row // 64)
SB_GROUP = 8                  # subblocks per DMA superblock
NSUPER = NSUB // SB_GROUP     # 8 superblocks
TOPK = 7

_CACHE = {}


def _dft_consts():
    """Packed-real DFT matrices W [512 feat, 512 packed] and C [512 packed, 512 t]."""
    j = np.arange(D)[:, None].astype(np.float64)
    f = np.arange(256)[None, :].astype(np.float64)
    Wc = np.cos(-2 * np.pi * j * f / D)
    Ws = np.sin(-2 * np.pi * j * f / D)
    WB = Ws.copy()
    WB[:, 0] = np.cos(np.pi * j[:, 0])          # B0 row: Re256
    W = np.concatenate([Wc, WB], axis=1)        # [512, 512]
    t = np.arange(D)[None, :].astype(np.float64)
    fc = np.arange(256)[:, None].astype(np.float64)
    Ca = np.cos(2 * np.pi * fc * t / D) * 2 / D
    Ca[0] = 1.0 / D
    Cb = -np.sin(2 * np.pi * fc * t / D) * 2 / D
    Cb[0] = np.cos(np.pi * t[0]) / D
    C = np.concatenate([Ca, Cb], axis=0)        # [512, 512]
    return W.astype(np.float32), C.astype(np.float32)


def _build_nc(n_iter=1, internal_io=False):
    import os
    import concourse.bacc as bacc
    import concourse.mybir as mybir
    from concourse.tile import TileContext

    f16 = mybir.dt.float16
    f32 = mybir.dt.float32

    W, C = _dft_consts()
    # W16[p, jj, fp]  = W[jj*128+p, fp]   (lhsT blocks for GEMM-1)
    W16 = W.reshape(4, P, D).transpose(1, 0, 2).astype(np.float16).copy()
    # C16[p, ff, t]   = C[ff*128+p, t]    (rhs blocks for GEMM-2)
    C16 = C.reshape(4, P, D).transpose(1, 0, 2).astype(np.float16).copy()
    ID16 = np.eye(P, dtype=np.float16)

    nc = bacc.Bacc()
    tick_d = tock_d = None
    if internal_io:
        # timing-only build: big tensors Internal (no host transfer), tiny ext io
        tick_d = nc.dram_tensor("tick", [1, 64], f32, kind="ExternalInput")
        tock_d = nc.dram_tensor("tock", [1, 64], f32, kind="ExternalOutput")
        q_d = nc.dram_tensor("query", [ROWS, D], f32, kind="Internal")
        k_d = nc.dram_tensor("key", [ROWS, D], f32, kind="Internal")
        v_d = nc.dram_tensor("value", [ROWS, D], f32, kind="Internal")
        o_d = nc.dram_tensor("out", [ROWS, D], f16, kind="Internal")
    else:
        q_d = nc.dram_tensor("query", [ROWS, D], f32, kind="ExternalInput")
        k_d = nc.dram_tensor("key", [ROWS, D], f32, kind="ExternalInput")
        v_d = nc.dram_tensor("value", [ROWS, D], f32, kind="ExternalInput")
        o_d = nc.dram_tensor("out", [ROWS, D], f16, kind="ExternalOutput")
    w_t = nc.inline_tensor(W16, name="Wdft")
    c_t = nc.inline_tensor(C16, name="Cdft")
    i_t = nc.inline_tensor(ID16, name="Ident")

    # interleaved views: [p, s, c] with row = 64*p + s
    qv = q_d.rearrange("(p s) c -> p s c", s=NSUB)
    kv = k_d.rearrange("(p s) c -> p s c", s=NSUB)
    vv = v_d.rearrange("(p s) c -> p s c", s=NSUB)
    ov = o_d.rearrange("(p s) c -> p s c", s=NSUB)

    with TileContext(nc) as tc:
        with (
            tc.tile_pool(name="consts", bufs=1) as consts,
            tc.tile_pool(name="io", bufs=2) as io,
            tc.tile_pool(name="work", bufs=3) as work,
            tc.tile_pool(name="small", bufs=8) as small,
            tc.tile_pool(name="ps", bufs=2, space="PSUM") as psp,
            tc.tile_pool(name="pscb", bufs=2, space="PSUM") as pscp,
            tc.tile_pool(name="psx", bufs=2, space="PSUM") as psxp,
        ):
            if internal_io:
                tkt = consts.tile([1, 64], f32)
                nc.sync.dma_start(out=tkt, in_=tick_d[:, :])
            wt = consts.tile([P, 4, D], f16)      # W16
            ct = consts.tile([P, 4, D], f16)      # C16
            idt = consts.tile([P, P], f16)        # identity for PE transpose
            nc.sync.dma_start(out=wt, in_=w_t[:, :, :])
            nc.sync.dma_start(out=ct, in_=c_t[:, :, :])
            nc.sync.dma_start(out=idt, in_=i_t[:, :])

            # vsh[p] = v[row 64p+64] ; fix wraps at p in {31,63,95,127} <- batch starts
            vsh = consts.tile([P, D], f16)
            vflat = v_d  # [ROWS, D]
            nc.gpsimd.dma_start(
                out=vsh[0:127], in_=vflat.rearrange("(a b) c -> a b c", b=NSUB)[1:128, 0]
            )  # rows 64,128,...,8128
            nc.gpsimd.dma_start(
                out=vsh.rearrange("(w u) c -> w u c", u=32)[:, 31:32, :].rearrange("w u c -> (w u) c"),
                in_=vflat.rearrange("(b t) c -> b t c", t=L)[:, 0:1, :].rearrange("b t c -> (b t) c"),
            )  # vsh[31,63,95,127] <- v rows {0, 2048, 4096, 6144}

            def load_super(sbi):
                sl = slice(sbi * SB_GROUP, (sbi + 1) * SB_GROUP)
                q16 = io.tile([P, SB_GROUP, D], f16, tag="q16")
                k16 = io.tile([P, SB_GROUP, D], f16, tag="k16")
                v16 = io.tile([P, SB_GROUP, D], f16, tag="v16")
                # casting DMAs must use the gpsimd (SWDGE) queue
                nc.gpsimd.dma_start(out=q16, in_=qv[:, sl, :])
                nc.gpsimd.dma_start(out=k16, in_=kv[:, sl, :])
                nc.gpsimd.dma_start(out=v16, in_=vv[:, sl, :])
                return q16, k16, v16

            def compute_group(q16, k16, gl, w1sb):
                """gl: local group index (0..3); transposes subblocks 2gl, 2gl+1
                of q16/k16 on the PE, then GEMM-1/products/GEMM-2; writes w1
                into w1sb[:, 2gl+sp, :]."""
                # PE transpose: chunk (u, jj) -> ptx[:, 2*jj+u, :] (f16 in PSUM)
                ptq = psxp.tile([P, 8, P], f16, tag="ptx")
                for u in range(2):
                    for jj in range(4):
                        nc.tensor.transpose(
                            ptq[:, 2 * jj + u, :],
                            q16[:, 2 * gl + u, jj * P:(jj + 1) * P], idt)
                qTg = work.tile([P, 8, P], f16, tag="qTg", bufs=2)
                nc.scalar.copy(qTg, ptq)
                ptk = psxp.tile([P, 8, P], f16, tag="ptx")
                for u in range(2):
                    for jj in range(4):
                        nc.tensor.transpose(
                            ptk[:, 2 * jj + u, :],
                            k16[:, 2 * gl + u, jj * P:(jj + 1) * P], idt)
                kTg = work.tile([P, 8, P], f16, tag="kTg", bufs=2)
                nc.scalar.copy(kTg, ptk)

                psq = psp.tile([P, 4, 256], f32, tag="ps2bank")
                psk = psp.tile([P, 4, 256], f32, tag="ps2bank")
                for mm in range(4):
                    for jj in range(4):
                        nc.tensor.matmul(psq[:, mm, :], wt[:, jj, mm * P:(mm + 1) * P],
                                         qTg[:, 2 * jj:2 * jj + 2, :],
                                         start=(jj == 0), stop=(jj == 3))
                for mm in range(4):
                    for jj in range(4):
                        nc.tensor.matmul(psk[:, mm, :], wt[:, jj, mm * P:(mm + 1) * P],
                                         kTg[:, 2 * jj:2 * jj + 2, :],
                                         start=(jj == 0), stop=(jj == 3))

                qf = work.tile([P, 4, 256], f16, tag="qf")
                kf = work.tile([P, 4, 256], f16, tag="kf")
                nc.scalar.copy(qf, psq)
                nc.scalar.copy(kf, psk)

                # products: Pa = QA.KA + QB.KB ; Pb = QB.KA - QA.KB
                pt = work.tile([P, 4, 256], f16, tag="pt")
                t1 = work.tile([P, 2, 256], f16, tag="t1")
                t2 = work.tile([P, 2, 256], f16, tag="t2")
                QA, QB = qf[:, 0:2, :], qf[:, 2:4, :]
                KA, KB = kf[:, 0:2, :], kf[:, 2:4, :]
                nc.vector.tensor_mul(t1, QA, KA)
                nc.vector.tensor_mul(t2, QB, KB)
                nc.vector.tensor_add(pt[:, 0:2, :], t1, t2)
                nc.vector.tensor_mul(t1, QB, KA)
                nc.vector.tensor_mul(t2, QA, KB)
                nc.vector.tensor_sub(pt[:, 2:4, :], t1, t2)
                # f=0 fixups (partition 0 of slices 0 and 2), one strided op
                nc.vector.tensor_mul(
                    pt[0:1, 0:4:2, :], qf[0:1, 0:4:2, :], kf[0:1, 0:4:2, :])

                for sp in range(2):
                    cps = pscp.tile([P, D], f32, tag="psc1bank")
                    for ff in range(4):
                        nc.tensor.matmul(cps, pt[:, ff, sp * P:(sp + 1) * P],
                                         ct[:, ff, :], start=(ff == 0), stop=(ff == 3))
                    mx = small.tile([P, 8], f32, tag="mx")
                    nc.vector.max(out=mx, in_=cps)
                    sm = small.tile([P, 1], f32, tag="sm")
                    nc.vector.reduce_sum(sm, mx[:, 0:TOPK], axis=mybir.AxisListType.X)
                    pm = small.tile([P, 1], f32, tag="pm")
                    nc.vector.tensor_scalar_mul(pm, sm, 1.0 / TOPK)
                    nc.scalar.activation(w1sb[:, 2 * gl + sp, :], cps,
                                         mybir.ActivationFunctionType.Sigmoid,
                                         bias=pm, scale=-1.0)

            def combine_super(v16, w1sb, vnext0, o16):
                """o16[:, s] = v16[:, s] + w1sb[:, s]*(v16[:, s+1] - v16[:, s]);
                s=7 uses vnext0."""
                for sl in range(SB_GROUP):
                    vnext = v16[:, sl + 1, :] if sl < SB_GROUP - 1 else vnext0
                    dt_ = work.tile([P, D], f16, tag="dt")
                    zt = work.tile([P, D], f16, tag="zt")
                    nc.vector.tensor_sub(dt_, vnext, v16[:, sl, :])
                    nc.vector.tensor_mul(zt, w1sb[:, sl, :], dt_)
                    nc.gpsimd.tensor_add(o16[:, sl, :], v16[:, sl, :], zt)

            def pipeline():
                prev = None  # (v16, o16, w1sb, sbi)
                for sbi in range(NSUPER):
                    q16, k16, v16 = load_super(sbi)
                    o16 = io.tile([P, SB_GROUP, D], f16, tag="o16")
                    w1sb = work.tile([P, SB_GROUP, D], f16, tag="w1sb", bufs=2)
                    for gl in range(4):
                        compute_group(q16, k16, gl, w1sb)
                    if prev is not None:
                        pv, po, pw, psbi = prev
                        combine_super(pv, pw, v16[:, 0, :], po)
                        nc.sync.dma_start(
                            out=ov[:, psbi * SB_GROUP:(psbi + 1) * SB_GROUP, :], in_=po)
                    prev = (v16, o16, w1sb, sbi)

                pv, po, pw, psbi = prev
                combine_super(pv, pw, vsh, po)
                nc.sync.dma_start(
                    out=ov[:, psbi * SB_GROUP:(psbi + 1) * SB_GROUP, :], in_=po)

            if n_iter == 1:
                pipeline()
            else:
                with tc.For_i(0, n_iter, 1):
                    pipeline()

            if internal_io:
                nc.sync.dma_start(out=tock_d[:, :], in_=tkt)

    nc.finalize()
    return nc


def kernel(query, key, value):
    import sys
    if "/opt/trn_rl_repo" not in sys.path:
        sys.path.insert(0, "/opt/trn_rl_repo")
    from concourse.bass_utils import run_bass_kernel_spmd

    if "nc" not in _CACHE:
        _CACHE["nc"] = _build_nc()
    nc = _CACHE["nc"]

    q = np.ascontiguousarray(np.asarray(query, dtype=np.float32).reshape(B, L, D))
    k = np.ascontiguousarray(np.asarray(key, dtype=np.float32).reshape(B, L, D))
    v = np.ascontiguousarray(np.asarray(value, dtype=np.float32).reshape(B, L, D))

    in_maps = []
    for c in range(N_CORES):
        sl = slice(c * BPC, (c + 1) * BPC)
        in_maps.append({
            "query": q[sl].reshape(ROWS, D),
            "key": k[sl].reshape(ROWS, D),
            "value": v[sl].reshape(ROWS, D),
        })
    res = run_bass_kernel_spmd(nc, in_maps, core_ids=list(range(N_CORES)))
    _CACHE["last_result"] = res
    out = np.empty((B, L, D), dtype=np.float32)
    for c in range(N_CORES):
        out[c * BPC:(c + 1) * BPC] = res.results[c]["out"].astype(np.float32).reshape(BPC, L, D)
    return out


# revision 3
# speedup vs baseline: 1.4784x; 1.0781x over previous
"""AutoCorrelation Trainium2 kernel (Bass/Tile, 8 NeuronCores) — v3.

Math (per row r of [B*L, 512] with D=512):
  corr_r = irfft(rfft(q_r) * conj(rfft(k_r)))            (circular cross-correlation)
  mean_r = mean(top7(corr_r))
  w0 = sigmoid(corr - mean); out = v*w0 + roll(v,-1,L)*(1-w0)
     = v + sigmoid(mean - corr) * (roll(v) - v)

v3: re-blocked for measured per-instruction overheads:
  - PE matmul ~= N*0.417ns + 15..50ns with enough PSUM banks; PSUM-slot reuse
    stalls (~120ns/mm) dominate at small N -> use N=512 4-accum chains and
    bank-alternating rings.
  - ACT op ~= 460ns fixed + 0.83ns/elem -> merge copies into wide [128,1024+]
    ops, split across ACT/DVE.
  - Pool ALU 1.73ns/elem -> only the final combine add/mul live there.
  - Transposes on PE (f16 is_transpose chunks into PSUM), NOT the DMA xbar:
    fabric cap ~320-340 GB/s/core is fully consumed by the mandatory HBM I/O.
  - Processing unit = half-superblock (4 subblocks = 512 rows).
"""
import numpy as np

B, L, D = 32, 2048, 512
N_CORES = 8
BPC = B // N_CORES            # batches per core
ROWS = BPC * L                # 8192 rows per core
NSUB = 64                     # subblocks (s = row % 64)
P = 128                       # partitions (p = row // 64)
SB_GROUP = 8                  # subblocks per DMA superblock
NSUPER = NSUB // SB_GROUP     # 8 superblocks
TOPK = 7

_CACHE = {}


def _dft_consts():
    """Packed-real DFT matrices W [512 feat, 512 packed] and C [512 packed, 512 t]."""
    j = np.arange(D)[:, None].astype(np.float64)
    f = np.arange(256)[None, :].astype(np.float64)
    Wc = np.cos(-2 * np.pi * j * f / D)
    Ws = np.sin(-2 * np.pi * j * f / D)
    WB = Ws.copy()
    WB[:, 0] = np.cos(np.pi * j[:, 0])          # B0 row: Re256
    W = np.concatenate([Wc, WB], axis=1)        # [512, 512]
    t = np.arange(D)[None, :].astype(np.float64)
    fc = np.arange(256)[:, None].astype(np.float64)
    Ca = np.cos(2 * np.pi * fc * t / D) * 2 / D
    Ca[0] = 1.0 / D
    Cb = -np.sin(2 * np.pi * fc * t / D) * 2 / D
    Cb[0] = np.cos(np.pi * t[0]) / D
    C = np.concatenate([Ca, Cb], axis=0)        # [512, 512]
    return W.astype(np.float32), C.astype(np.float32)


def _build_nc(n_iter=1, internal_io=False):
    import concourse.bacc as bacc
    import concourse.mybir as mybir
    from concourse.tile import TileContext

    f16 = mybir.dt.float16
    f32 = mybir.dt.float32

    W, C = _dft_consts()
    # W16[p, jj, fp]  = W[jj*128+p, fp]   (lhsT blocks for GEMM-1)
    W16 = W.reshape(4, P, D).transpose(1, 0, 2).astype(np.float16).copy()
    # C16[p, ff, t]   = C[ff*128+p, t]    (rhs blocks for GEMM-2)
    C16 = C.reshape(4, P, D).transpose(1, 0, 2).astype(np.float16).copy()
    ID16 = np.eye(P, dtype=np.float16)

    nc = bacc.Bacc()
    tick_d = tock_d = None
    if internal_io:
        tick_d = nc.dram_tensor("tick", [1, 64], f32, kind="ExternalInput")
        tock_d = nc.dram_tensor("tock", [1, 64], f32, kind="ExternalOutput")
        q_d = nc.dram_tensor("query", [ROWS, D], f32, kind="Internal")
        k_d = nc.dram_tensor("key", [ROWS, D], f32, kind="Internal")
        v_d = nc.dram_tensor("value", [ROWS, D], f32, kind="Internal")
        o_d = nc.dram_tensor("out", [ROWS, D], f16, kind="Internal")
    else:
        q_d = nc.dram_tensor("query", [ROWS, D], f32, kind="ExternalInput")
        k_d = nc.dram_tensor("key", [ROWS, D], f32, kind="ExternalInput")
        v_d = nc.dram_tensor("value", [ROWS, D], f32, kind="ExternalInput")
        o_d = nc.dram_tensor("out", [ROWS, D], f16, kind="ExternalOutput")
    w_t = nc.inline_tensor(W16, name="Wdft")
    c_t = nc.inline_tensor(C16, name="Cdft")
    i_t = nc.inline_tensor(ID16, name="Ident")

    qv = q_d.rearrange("(p s) c -> p s c", s=NSUB)
    kv = k_d.rearrange("(p s) c -> p s c", s=NSUB)
    vv = v_d.rearrange("(p s) c -> p s c", s=NSUB)
    ov = o_d.rearrange("(p s) c -> p s c", s=NSUB)

    with TileContext(nc) as tc:
        with (
            tc.tile_pool(name="consts", bufs=1) as consts,
            tc.tile_pool(name="io", bufs=2) as io,
            tc.tile_pool(name="work", bufs=2) as work,
            tc.tile_pool(name="small", bufs=8) as small,
            tc.tile_pool(name="psx", bufs=3, space="PSUM") as psxp,   # 3x1 bank
            tc.tile_pool(name="psg1", bufs=3, space="PSUM") as psg1,  # 3x1 bank
            tc.tile_pool(name="psg2", bufs=2, space="PSUM") as psg2,  # 2x1 bank
        ):
            if internal_io:
                tkt = consts.tile([1, 64], f32)
                nc.sync.dma_start(out=tkt, in_=tick_d[:, :])
            wt = consts.tile([P, 4, D], f16)      # W16
            ct = consts.tile([P, 4, D], f16)      # C16
            idt = consts.tile([P, P], f16)        # identity for PE transpose
            nc.sync.dma_start(out=wt, in_=w_t[:, :, :])
            nc.sync.dma_start(out=ct, in_=c_t[:, :, :])
            nc.sync.dma_start(out=idt, in_=i_t[:, :])

            # vsh[p] = v[row 64p+64] ; wraps at p in {31,63,95,127} <- batch starts
            vsh = consts.tile([P, D], f16)
            vflat = v_d
            nc.gpsimd.dma_start(
                out=vsh[0:127], in_=vflat.rearrange("(a b) c -> a b c", b=NSUB)[1:128, 0]
            )
            nc.gpsimd.dma_start(
                out=vsh.rearrange("(w u) c -> w u c", u=32)[:, 31:32, :].rearrange("w u c -> (w u) c"),
                in_=vflat.rearrange("(b t) c -> b t c", t=L)[:, 0:1, :].rearrange("b t c -> (b t) c"),
            )

            def load_super(sbi):
                sl = slice(sbi * SB_GROUP, (sbi + 1) * SB_GROUP)
                q16 = io.tile([P, SB_GROUP, D], f16, tag="q16")
                k16 = io.tile([P, SB_GROUP, D], f16, tag="k16")
                v16 = io.tile([P, SB_GROUP, D], f16, tag="v16")
                nc.gpsimd.dma_start(out=q16, in_=qv[:, sl, :])
                nc.gpsimd.dma_start(out=k16, in_=kv[:, sl, :])
                nc.gpsimd.dma_start(out=v16, in_=vv[:, sl, :])
                return q16, k16, v16

            def compute_half(q16, k16, hh, w1sb):
                """hh in {0,1}: subblocks u0=4*hh .. u0+3 (512 rows).

                xpose chunks (t, u, jj) on PE -> ptx PSUM f16 -> qkT SBUF;
                GEMM-1 4-jj-accum chains N=512 -> psA/psB -> qkf SBUF f16;
                products on DVE -> pt; GEMM-2 4-ff chains N=512 per subblock;
                max8 + sigmoid -> w1sb[:, u, :].
                """
                u0 = 4 * hh
                # ---- transpose q,k chunks: ptx tile = (jj-half jh: 2 jj) x 4 u
                qkT = work.tile([P, 2, 4, 4, P], f16, tag="qkT", bufs=3)  # [t, jj, u, 128]
                for t, x16 in ((0, q16), (1, k16)):
                    for jh in range(2):
                        ptx = psxp.tile([P, 8, P], f16, tag="ptx")
                        # interleave the two jj of this half to alternate offsets
                        for u in range(4):
                            for j2 in range(2):
                                jj = 2 * jh + j2
                                nc.tensor.transpose(
                                    ptx[:, 4 * j2 + u, :],
                                    x16[:, u0 + u, jj * P:(jj + 1) * P], idt)
                        # copy ptx -> qkT[:, t, 2jh:2jh+2, :, :]
                        dst = qkT[:, t, 2 * jh:2 * jh + 2, :, :]
                        if (t + jh) % 2 == 0:
                            nc.scalar.copy(dst, ptx.rearrange("p (a b) c -> p a b c", a=2))
                        else:
                            nc.vector.tensor_copy(dst, ptx.rearrange("p (a b) c -> p a b c", a=2))

                # ---- GEMM-1: chain over jj accumulating, N=512, 1-bank tiles
                qkf = work.tile([P, 2, 2, 2, D], f16, tag="qkf", bufs=3)  # [t, ab, m2, 512]
                for t in range(2):
                    for ab in range(2):
                        for m2 in range(2):
                            mm = 2 * ab + m2
                            ps = psg1.tile([P, D], f32, tag="g1")
                            for jj in range(4):
                                nc.tensor.matmul(
                                    ps,
                                    wt[:, jj, mm * P:(mm + 1) * P],
                                    qkT[:, t, jj, :, :],
                                    start=(jj == 0), stop=(jj == 3))
                            dst = qkf[:, t, ab, m2, :]
                            if (t + ab + m2) % 2 == 0:
                                nc.scalar.copy(dst, ps)
                            else:
                                nc.vector.tensor_copy(dst, ps)

                # ---- products: Pa = QA.KA + QB.KB ; Pb = QB.KA - QA.KB
                pt = work.tile([P, 4, D], f16, tag="pt", bufs=3)       # [mm, 512 rows]
                t1 = work.tile([P, 2, D], f16, tag="t1")
                t2 = work.tile([P, 2, D], f16, tag="t2")
                QA, QB = qkf[:, 0, 0, :, :], qkf[:, 0, 1, :, :]
                KA, KB = qkf[:, 1, 0, :, :], qkf[:, 1, 1, :, :]
                nc.vector.tensor_mul(t1, QA, KA)
                nc.vector.tensor_mul(t2, QB, KB)
                nc.vector.tensor_add(pt[:, 0:2, :], t1, t2)
                nc.vector.tensor_mul(t1, QB, KA)
                nc.vector.tensor_mul(t2, QA, KB)
                nc.vector.tensor_sub(pt[:, 2:4, :], t1, t2)
                # f=0 fixup: partition 0 of mm0 (Re0) and mm2 (Re256)
                nc.vector.tensor_mul(
                    pt[0:1, 0:4:2, :], qkf[0:1, 0, :, 0, :], qkf[0:1, 1, :, 0, :])

                # ---- GEMM-2 + top7-mean + sigmoid per subblock (128 rows)
                for u in range(4):
                    cps = psg2.tile([P, D], f32, tag="g2")
                    for ff in range(4):
                        nc.tensor.matmul(cps, pt[:, ff, u * P:(u + 1) * P],
                                         ct[:, ff, :], start=(ff == 0), stop=(ff == 3))
                    mx = small.tile([P, 8], f32, tag="mx")
                    nc.vector.max(out=mx, in_=cps)
                    sm = small.tile([P, 1], f32, tag="sm")
                    nc.vector.reduce_sum(sm, mx[:, 0:TOPK], axis=mybir.AxisListType.X)
                    pm = small.tile([P, 1], f32, tag="pm")
                    nc.vector.tensor_scalar_mul(pm, sm, 1.0 / TOPK)
                    nc.scalar.activation(w1sb[:, u0 + u, :], cps,
                                         mybir.ActivationFunctionType.Sigmoid,
                                         bias=pm, scale=-1.0)

            def combine_super(v16, w1sb, vnext0, o16):
                """o16 = v16 + w1sb*(roll(v16) - v16), wide ops."""
                dtw = work.tile([P, SB_GROUP, D], f16, tag="dtw")
                ztw = work.tile([P, SB_GROUP, D], f16, tag="ztw")
                nc.vector.tensor_sub(dtw[:, 0:7, :], v16[:, 1:8, :], v16[:, 0:7, :])
                nc.vector.tensor_sub(dtw[:, 7, :], vnext0, v16[:, 7, :])
                # keep Pool free: it must pump the casting load DMAs
                nc.vector.tensor_mul(ztw, w1sb, dtw)
                nc.vector.tensor_add(o16, v16, ztw)

            def pipeline():
                prev = None  # (v16, o16, w1sb, sbi)
                for sbi in range(NSUPER):
                    q16, k16, v16 = load_super(sbi)
                    o16 = io.tile([P, SB_GROUP, D], f16, tag="o16")
                    w1sb = work.tile([P, SB_GROUP, D], f16, tag="w1sb")
                    for hh in range(2):
                        compute_half(q16, k16, hh, w1sb)
                    if prev is not None:
                        pv, po, pw, psbi = prev
                        combine_super(pv, pw, v16[:, 0, :], po)
                        nc.sync.dma_start(
                            out=ov[:, psbi * SB_GROUP:(psbi + 1) * SB_GROUP, :], in_=po)
                    prev = (v16, o16, w1sb, sbi)

                pv, po, pw, psbi = prev
                combine_super(pv, pw, vsh, po)
                nc.sync.dma_start(
                    out=ov[:, psbi * SB_GROUP:(psbi + 1) * SB_GROUP, :], in_=po)

            if n_iter == 1:
                pipeline()
            else:
                with tc.For_i(0, n_iter, 1):
                    pipeline()

            if internal_io:
                nc.sync.dma_start(out=tock_d[:, :], in_=tkt)

    nc.finalize()
    return nc


def kernel(query, key, value):
    import sys
    if "/opt/trn_rl_repo" not in sys.path:
        sys.path.insert(0, "/opt/trn_rl_repo")
    from concourse.bass_utils import run_bass_kernel_spmd

    if "nc" not in _CACHE:
        _CACHE["nc"] = _build_nc()
    nc = _CACHE["nc"]

    q = np.ascontiguousarray(np.asarray(query, dtype=np.float32).reshape(B, L, D))
    k = np.ascontiguousarray(np.asarray(key, dtype=np.float32).reshape(B, L, D))
    v = np.ascontiguousarray(np.asarray(value, dtype=np.float32).reshape(B, L, D))

    in_maps = []
    for c in range(N_CORES):
        sl = slice(c * BPC, (c + 1) * BPC)
        in_maps.append({
            "query": q[sl].reshape(ROWS, D),
            "key": k[sl].reshape(ROWS, D),
            "value": v[sl].reshape(ROWS, D),
        })
    res = run_bass_kernel_spmd(nc, in_maps, core_ids=list(range(N_CORES)))
    _CACHE["last_result"] = res
    out = np.empty((B, L, D), dtype=np.float32)
    for c in range(N_CORES):
        out[c * BPC:(c + 1) * BPC] = res.results[c]["out"].astype(np.float32).reshape(BPC, L, D)
    return out
